# revision 21
# baseline (speedup 1.0000x reference)
"""NF4-quantized linear layer (x @ dequant(W).T + dequant(b)) on 8 Trainium2 cores.

Strategy (column-parallel / tensor-parallel):
  - Shard the out_features dim (14336) into 8 shards of 1792; replicate x.
  - Host side: fully dequantize W (NF4 code lookup * per-64-block absmax) in
    f32, round once to bf16, pre-transposed into W.T layout; pre-transpose x
    into x.T tiles; fully dequantize the bias.
  - Device side (per core): stream W.T into a resident SBUF tile on the ACT
    HWDGE ring (one clean 448KB DMA per k-tile), run the tiled bf16 matmul
    with fp32 PSUM accumulation, add bias via DVE, stream results out on the
    SP ring.  The first two m-tiles run k-major so the PE has 8 PSUM
    accumulation groups to feed from each newly arrived W k-tile — the PE
    saturates ~4us in and the HAM clock never re-throttles.
  - Gather: concatenate the 8 output shards on the feature axis.
"""

import sys

sys.path.insert(0, "/opt/trn_rl_repo")

import numpy as np
import ml_dtypes

import concourse.bass as bass
import concourse.tile as tile
from concourse import mybir
from concourse.vector_clock import ScopedClock
from concourse.bass_utils import run_bass_kernel_spmd

BF16 = ml_dtypes.bfloat16

OUT_F = 14336
IN_F = 4096
M_ROWS = 8192
BLOCK = 64
N_CORES = 8
SHARD = OUT_F // N_CORES  # 1792

K_TILES = IN_F // 128  # 32
M_TILES = M_ROWS // 128  # 64
N_CHUNKS = [(0, 512), (512, 512), (1024, 512), (1536, 256)]

NF4 = np.array(
    [
        -1.0, -0.6961928009986877, -0.5250730514526367, -0.39491748809814453,
        -0.28444138169288635, -0.18477343022823334, -0.09105003625154495, 0.0,
        0.07958029955625534, 0.16093020141124725, 0.24611230194568634,
        0.33791524171829224, 0.44070982933044434, 0.5626170039176941,
        0.7229568362236023, 1.0,
    ],
    dtype=np.float32,
)


def _patched_drain_and_barrier(self, tick_clock, wait_clock):
    # This walrus build rejects >1 sync-wait on the SP/CTRL-queue drain that
    # Tile emits at kernel tail ("Too many sync wait commands").  Split the
    # waits across extra no-ops, one wait each.
    drain_inst = self.nc.sync.drain()
    wait_clock.add_sem_waits(
        drain_inst.ins, ScopedClock({None: tick_clock.global_clock})
    )
    waits = list(drain_inst.ins.sync_info.on_wait or [])
    if len(waits) > 1:
        drain_inst.ins.sync_info.on_wait = waits[:1]
        for i in range(1, len(waits)):
            nop = self.nc.sync.nop(nofuse=True)
            nop.ins.sync_info = mybir.SyncInfo(on_wait=waits[i : i + 1], on_update=[])
    self.nc.all_engine_barrier()
    assert self.sems is not None
    popped = self.nc._tile_sem_poison_stack.pop()
    assert popped is self._sem_poison
    self.nc.clear_and_free_semaphores(list(self.sems.allocated().values()))
    self.nc.all_engine_barrier()


tile.TileContext._drain_and_barrier = _patched_drain_and_barrier


import concourse.tile_sem_assignment as _tsa
from concourse import bass_isa as _bass_isa

_orig_assign_tick = _tsa.TileClockTick._assign_tick


def _assign_tick_per_ring(self, inst):
    """Partition the 8 HWDGE completion-sem lanes by issuing ring (SP ring
    lanes 0-3, ACT ring lanes 4-7).  Stock Tile round-robins one shared
    counter across both rings, but HWDGE DMAs only complete in FIFO order
    per ring — a later fast DMA on the other ring can hit a shared lane's
    threshold while an earlier DMA's data is still in flight.  Per-ring
    lanes make every lane's increments monotone in tick order."""
    if (
        isinstance(inst, _tsa.DMAInst)
        and not isinstance(inst, _bass_isa.UserSyncedRemoteDMADescs)
        and inst.engine != mybir.EngineType.Pool
    ):
        ring = 0 if inst.engine == mybir.EngineType.SP else 1
        ctr = getattr(self, "_ring_hw_ctr", None)
        if ctr is None:
            ctr = self._ring_hw_ctr = [0, 4]
        self.next_hw_dma_idx = ctr[ring]
        _orig_assign_tick(self, inst)
        base = 0 if ring == 0 else 4
        ctr[ring] = base + (ctr[ring] - base + 1) % 4
    else:
        _orig_assign_tick(self, inst)


_tsa.TileClockTick._assign_tick = _assign_tick_per_ring


def _split_multi_waits(nc, max_waits=1):
    """This walrus build accepts at most one sync-wait per instruction.
    Move extra waits onto same-engine no-ops inserted just before the
    instruction (engine queues are in-order, so semantics are unchanged)."""
    n = 0
    for f in nc.m.functions:
        for bb in f.blocks:
            out_list = []
            for ins in bb.instructions:
                si = getattr(ins, "sync_info", None)
                waits = list(si.on_wait) if si is not None and si.on_wait else []
                if len(waits) > max_waits:
                    for w in waits[: len(waits) - max_waits]:
                        nop = mybir.InstNoOp(
                            name=f"I-waitsplit-{n}",
                            ins=[],
                            outs=[],
                            engine=ins.engine,
                            sync_info=mybir.SyncInfo(on_wait=[w], on_update=[]),
                        )
                        n += 1
                        out_list.append(nop)
                    si.on_wait = waits[len(waits) - max_waits :]
                out_list.append(ins)
            bb.instructions[:] = out_list
    return n


def _build_program(m_tiles=M_TILES, split_waits=True):
    nc = bass.Bass("TRN2", target_bir_lowering=False, debug=False, num_devices=1)

    wq = nc.dram_tensor("wq", [IN_F, SHARD], mybir.dt.bfloat16, kind="ExternalInput").ap()
    xt = nc.dram_tensor("xt", [m_tiles, 128, K_TILES, 128], mybir.dt.bfloat16, kind="ExternalInput").ap()
    bias = nc.dram_tensor("bias", [1, SHARD], mybir.dt.bfloat16, kind="ExternalInput").ap()
    out = nc.dram_tensor("out", [m_tiles * 128, SHARD], mybir.dt.float32, kind="ExternalOutput").ap()

    with tile.TileContext(nc) as tc:
        with (
            tc.tile_pool(name="wres", bufs=1) as wres_pool,
            tc.tile_pool(name="bstg", bufs=1) as bstg_pool,
            tc.tile_pool(name="bias", bufs=1) as bias_pool,
            tc.tile_pool(name="xin", bufs=5) as x_pool,
            tc.tile_pool(name="oput", bufs=6) as o_pool,
            tc.tile_pool(name="psum", bufs=8, space="PSUM") as ps_pool,
        ):
            # Resident pre-scaled weights: W.T layout, k-tile t at cols
            # [t*SHARD, (t+1)*SHARD)
            wsc = wres_pool.tile([128, K_TILES * SHARD], mybir.dt.bfloat16)

            # Startup is latency-critical AND bandwidth-critical: the first
            # matmul needs W k-tile 0 plus x0/x1's first k-chunk, and every
            # byte moved during the ~50us weight stream delays later k-tiles
            # (the head's PE work is exactly arrival-paced).  So the SP ring
            # leads with k0 then quarter-tiles of x0/x1 and goes idle, the
            # ACT ring streams k-tiles 1..31 at full HBM rate, and the
            # late-needed loads (bias, x2, x3) are held back by artificial
            # WAR deps on DVE "touch" ops that fire only when late k-tiles
            # land — keeping them out of the weight-stream window.
            X_PREFETCH = min(4, m_tiles)
            x_tiles = []
            for m in range(X_PREFETCH):
                xts = x_pool.tile([128, IN_F], mybir.dt.bfloat16, tag="xts", name=f"xts{m}")
                x_tiles.append(xts)
            nc.sync.dma_start(wsc[:, 0:SHARD], wq[0:128, :])
            for g in range(4):
                for m in range(min(2, m_tiles)):
                    nc.sync.dma_start(
                        x_tiles[m][:, g * 1024 : (g + 1) * 1024],
                        xt[m][:, g * 8 : (g + 1) * 8, :].rearrange("p t j -> p (t j)"),
                    )

            for t in range(1, K_TILES):
                nc.scalar.dma_start(
                    wsc[:, t * SHARD : (t + 1) * SHARD],
                    wq[t * 128 : (t + 1) * 128, :],
                )

            # Bias: replicate a 3.6KB bf16 row across partitions with a K=1
            # matmul against a ones-vector (PE is idle waiting for k0 anyway)
            # instead of a 0.9MB HBM broadcast DMA inside the weight window.
            ones_sb = bstg_pool.tile([1, 128], mybir.dt.bfloat16, tag="ones")
            bias_row = bstg_pool.tile([1, SHARD], mybir.dt.bfloat16, tag="brow")
            nc.vector.memset(ones_sb[:], 1.0)
            nc.sync.dma_start(bias_row[:], bias[:])
            bias_sb = bias_pool.tile([128, SHARD], mybir.dt.float32)
            for ic, (n0, nw) in enumerate(N_CHUNKS):
                ps_b = ps_pool.tile([128, 512], mybir.dt.float32, tag="ps", name=f"psb{ic}")
                nc.tensor.matmul(
                    ps_b[:, :nw],
                    lhsT=ones_sb[:, 0:128],
                    rhs=bias_row[:, n0 : n0 + nw],
                    start=True,
                    stop=True,
                )
                nc.vector.tensor_copy(bias_sb[:, n0 : n0 + nw], ps_b[:, :nw])

            # Touch ops: read one column of a late W k-tile (RAW on its DMA)
            # and scribble into the target tile (WAR gates the tile's DMA).
            # The DVE queue is otherwise idle until the head drains, so x2/x3
            # chunks transfer just-in-time as the weight stream finishes,
            # keeping the bytes-bound weight window clean.
            if m_tiles > 2:
                for g, kt in enumerate((28, 29, 30, 31)):
                    nc.vector.tensor_copy(
                        x_tiles[2][:, g * 1024 : g * 1024 + 1],
                        wsc[:, kt * SHARD : kt * SHARD + 1],
                    )
                    nc.sync.dma_start(
                        x_tiles[2][:, g * 1024 : (g + 1) * 1024],
                        xt[2][:, g * 8 : (g + 1) * 8, :].rearrange("p t j -> p (t j)"),
                    )
            if m_tiles > 3:
                nc.vector.tensor_copy(
                    x_tiles[3][:, 0:1], wsc[:, 30 * SHARD : 30 * SHARD + 1]
                )
                nc.sync.dma_start(x_tiles[3][:], xt[3].rearrange("p t j -> p (t j)"))

            def finish_tile(m, n0, nw, ps):
                ot = o_pool.tile([128, 512], mybir.dt.float32, tag="ot", name=f"ot{m}_{n0}")
                nc.vector.tensor_add(ot[:, :nw], ps[:, :nw], bias_sb[:, n0 : n0 + nw])
                nc.sync.dma_start(
                    out[m * 128 : (m + 1) * 128, n0 : n0 + nw], ot[:, :nw]
                )

            # First two m-tiles in k-major order: each newly arrived W k-tile
            # immediately feeds 8 PSUM accumulation groups (~1.5us of PE work
            # per ~1.3us k-tile DMA), so the PE saturates from the start.
            m_head = min(2, m_tiles)
            head_ps = {}
            for m in range(m_head):
                for ic, (n0, nw) in enumerate(N_CHUNKS):
                    head_ps[m, ic] = ps_pool.tile(
                        [128, 512], mybir.dt.float32, tag="ps", name=f"ps{m}_{ic}"
                    )
            for t in range(K_TILES):
                for m in range(m_head):
                    for ic, (n0, nw) in enumerate(N_CHUNKS):
                        nc.tensor.matmul(
                            head_ps[m, ic][:, :nw],
                            lhsT=x_tiles[m][:, t * 128 : (t + 1) * 128],
                            rhs=wsc[:, t * SHARD + n0 : t * SHARD + n0 + nw],
                            start=(t == 0),
                            stop=(t == K_TILES - 1),
                        )
            for m in range(m_head):
                for ic, (n0, nw) in enumerate(N_CHUNKS):
                    finish_tile(m, n0, nw, head_ps[m, ic])

            # Remaining m-tiles in m-major order
            for m in range(m_head, m_tiles):
                if m < X_PREFETCH:
                    xts = x_tiles[m]
                else:
                    xts = x_pool.tile([128, IN_F], mybir.dt.bfloat16, tag="xts", name=f"xts{m}")
                    nc.sync.dma_start(xts[:], xt[m].rearrange("p t j -> p (t j)"))
                for n0, nw in N_CHUNKS:
                    ps = ps_pool.tile([128, 512], mybir.dt.float32, tag="ps")
                    for t in range(K_TILES):
                        nc.tensor.matmul(
                            ps[:, :nw],
                            lhsT=xts[:, t * 128 : (t + 1) * 128],
                            rhs=wsc[:, t * SHARD + n0 : t * SHARD + n0 + nw],
                            start=(t == 0),
                            stop=(t == K_TILES - 1),
                        )
                    ot = o_pool.tile([128, 512], mybir.dt.float32, tag="ot")
                    nc.vector.tensor_add(ot[:, :nw], ps[:, :nw], bias_sb[:, n0 : n0 + nw])
                    nc.sync.dma_start(
                        out[m * 128 : (m + 1) * 128, n0 : n0 + nw], ot[:, :nw]
                    )

    if split_waits:
        _split_multi_waits(nc)
    return nc


_PROGRAM = None


def _get_program():
    global _PROGRAM
    if _PROGRAM is None:
        _PROGRAM = _build_program()
    return _PROGRAM


def _prep_inputs(x, w_packed, w_absmax, b_packed, b_absmax):
    """Host-side marshalling: full NF4 dequant (lookup + absmax scaling),
    layout transposes, sharding."""
    # Weights: packed int32 bytes -> W.T [IN_F, OUT_F] of scaled values.
    b = np.asarray(w_packed).astype(np.uint8).reshape(OUT_F, IN_F // 2)
    bT = np.ascontiguousarray(b.T)  # [2048, 14336]
    valsT = np.empty((IN_F, OUT_F), dtype=np.float32)
    valsT[0::2] = NF4[bT >> 4]
    valsT[1::2] = NF4[bT & 15]

    # Per-64-block absmax scaling: absmax[n, k//64] applied along k.
    am = np.asarray(w_absmax, dtype=np.float32).reshape(OUT_F, IN_F // BLOCK)
    wT = (
        valsT.reshape(IN_F // BLOCK, BLOCK, OUT_F) * am.T[:, None, :]
    ).reshape(IN_F, OUT_F).astype(BF16)

    # x: [M, K] f32 -> bf16 tiles [m_tile, p(k%128), k_tile, j(m%128)]
    xbf = np.asarray(x, dtype=np.float32).astype(BF16)
    xt5 = np.ascontiguousarray(
        xbf.reshape(M_TILES, 128, K_TILES, 128).transpose(0, 3, 2, 1)
    )

    # Bias: full dequant on host (14336 elements — negligible)
    bb = np.asarray(b_packed).astype(np.uint8)
    bcodes = np.empty(OUT_F, dtype=np.uint8)
    bcodes[0::2] = bb >> 4
    bcodes[1::2] = bb & 15
    bias_full = (
        NF4[bcodes].reshape(-1, BLOCK)
        * np.asarray(b_absmax, dtype=np.float32).reshape(-1, 1)
    ).reshape(OUT_F)

    in_maps = []
    for c in range(N_CORES):
        n0, n1 = c * SHARD, (c + 1) * SHARD
        in_maps.append(
            {
                "wq": np.ascontiguousarray(wT[:, n0:n1]),
                "xt": xt5,
                "bias": np.ascontiguousarray(bias_full[n0:n1]).astype(BF16).reshape(1, SHARD),
            }
        )
    return in_maps


def kernel(x, w_packed, w_absmax, b_packed, b_absmax, trace=False, **run_kwargs):
    nc = _get_program()
    in_maps = _prep_inputs(x, w_packed, w_absmax, b_packed, b_absmax)
    res = run_bass_kernel_spmd(
        nc, in_maps, core_ids=list(range(N_CORES)), trace=trace, **run_kwargs
    )
    out = np.concatenate([res.results[c]["out"] for c in range(N_CORES)], axis=1)
    kernel.last_results = res
    return out


# revision 23
# speedup vs baseline: 1.0112x; 1.0112x over previous
"""NF4-quantized linear layer (x @ dequant(W).T + dequant(b)) on 8 Trainium2 cores.

Strategy (column-parallel / tensor-parallel):
  - Shard the out_features dim (14336) into 8 shards of 1792; replicate x.
  - Host side: fully dequantize W (NF4 code lookup * per-64-block absmax) in
    f32, round once to bf16, pre-transposed into W.T layout; pre-transpose x
    into x.T tiles; fully dequantize the bias.
  - Device side (per core): stream W.T into a resident SBUF tile on the ACT
    HWDGE ring (one clean 448KB DMA per k-tile), run the tiled bf16 matmul
    with fp32 PSUM accumulation, add bias via DVE, stream results out on the
    SP ring.  The first two m-tiles run k-major so the PE has 8 PSUM
    accumulation groups to feed from each newly arrived W k-tile — the PE
    saturates ~4us in and the HAM clock never re-throttles.
  - Gather: concatenate the 8 output shards on the feature axis.
"""

import sys

sys.path.insert(0, "/opt/trn_rl_repo")

import numpy as np
import ml_dtypes

import concourse.bass as bass
import concourse.tile as tile
from concourse import mybir
from concourse.vector_clock import ScopedClock
from concourse.bass_utils import run_bass_kernel_spmd

BF16 = ml_dtypes.bfloat16

OUT_F = 14336
IN_F = 4096
M_ROWS = 8192
BLOCK = 64
N_CORES = 8
SHARD = OUT_F // N_CORES  # 1792

K_TILES = IN_F // 128  # 32
M_TILES = M_ROWS // 128  # 64
N_CHUNKS = [(0, 512), (512, 512), (1024, 512), (1536, 256)]

NF4 = np.array(
    [
        -1.0, -0.6961928009986877, -0.5250730514526367, -0.39491748809814453,
        -0.28444138169288635, -0.18477343022823334, -0.09105003625154495, 0.0,
        0.07958029955625534, 0.16093020141124725, 0.24611230194568634,
        0.33791524171829224, 0.44070982933044434, 0.5626170039176941,
        0.7229568362236023, 1.0,
    ],
    dtype=np.float32,
)


def _patched_drain_and_barrier(self, tick_clock, wait_clock):
    # This walrus build rejects >1 sync-wait on the SP/CTRL-queue drain that
    # Tile emits at kernel tail ("Too many sync wait commands").  Split the
    # waits across extra no-ops, one wait each.
    drain_inst = self.nc.sync.drain()
    wait_clock.add_sem_waits(
        drain_inst.ins, ScopedClock({None: tick_clock.global_clock})
    )
    waits = list(drain_inst.ins.sync_info.on_wait or [])
    if len(waits) > 1:
        drain_inst.ins.sync_info.on_wait = waits[:1]
        for i in range(1, len(waits)):
            nop = self.nc.sync.nop(nofuse=True)
            nop.ins.sync_info = mybir.SyncInfo(on_wait=waits[i : i + 1], on_update=[])
    self.nc.all_engine_barrier()
    assert self.sems is not None
    popped = self.nc._tile_sem_poison_stack.pop()
    assert popped is self._sem_poison
    self.nc.clear_and_free_semaphores(list(self.sems.allocated().values()))
    self.nc.all_engine_barrier()


tile.TileContext._drain_and_barrier = _patched_drain_and_barrier


import concourse.tile_sem_assignment as _tsa
from concourse import bass_isa as _bass_isa

_orig_assign_tick = _tsa.TileClockTick._assign_tick


def _assign_tick_per_ring(self, inst):
    """Partition the 8 HWDGE completion-sem lanes by issuing ring (SP ring
    lanes 0-3, ACT ring lanes 4-7).  Stock Tile round-robins one shared
    counter across both rings, but HWDGE DMAs only complete in FIFO order
    per ring — a later fast DMA on the other ring can hit a shared lane's
    threshold while an earlier DMA's data is still in flight.  Per-ring
    lanes make every lane's increments monotone in tick order."""
    if (
        isinstance(inst, _tsa.DMAInst)
        and not isinstance(inst, _bass_isa.UserSyncedRemoteDMADescs)
        and inst.engine != mybir.EngineType.Pool
    ):
        ring = 0 if inst.engine == mybir.EngineType.SP else 1
        ctr = getattr(self, "_ring_hw_ctr", None)
        if ctr is None:
            ctr = self._ring_hw_ctr = [0, 4]
        self.next_hw_dma_idx = ctr[ring]
        _orig_assign_tick(self, inst)
        base = 0 if ring == 0 else 4
        ctr[ring] = base + (ctr[ring] - base + 1) % 4
    else:
        _orig_assign_tick(self, inst)


_tsa.TileClockTick._assign_tick = _assign_tick_per_ring


def _split_multi_waits(nc, max_waits=1):
    """This walrus build accepts at most one sync-wait per instruction.
    Move extra waits onto same-engine no-ops inserted just before the
    instruction (engine queues are in-order, so semantics are unchanged)."""
    n = 0
    for f in nc.m.functions:
        for bb in f.blocks:
            out_list = []
            for ins in bb.instructions:
                si = getattr(ins, "sync_info", None)
                waits = list(si.on_wait) if si is not None and si.on_wait else []
                if len(waits) > max_waits:
                    for w in waits[: len(waits) - max_waits]:
                        nop = mybir.InstNoOp(
                            name=f"I-waitsplit-{n}",
                            ins=[],
                            outs=[],
                            engine=ins.engine,
                            sync_info=mybir.SyncInfo(on_wait=[w], on_update=[]),
                        )
                        n += 1
                        out_list.append(nop)
                    si.on_wait = waits[len(waits) - max_waits :]
                out_list.append(ins)
            bb.instructions[:] = out_list
    return n


def _build_program(m_tiles=M_TILES, split_waits=True):
    nc = bass.Bass("TRN2", target_bir_lowering=False, debug=False, num_devices=1)

    wq = nc.dram_tensor("wq", [IN_F, SHARD], mybir.dt.bfloat16, kind="ExternalInput").ap()
    xt = nc.dram_tensor("xt", [m_tiles, 128, K_TILES, 128], mybir.dt.bfloat16, kind="ExternalInput").ap()
    bias = nc.dram_tensor("bias", [1, SHARD], mybir.dt.bfloat16, kind="ExternalInput").ap()
    out = nc.dram_tensor("out", [m_tiles * 128, SHARD], mybir.dt.float32, kind="ExternalOutput").ap()

    with tile.TileContext(nc) as tc:
        with (
            tc.tile_pool(name="wres", bufs=1) as wres_pool,
            tc.tile_pool(name="bstg", bufs=1) as bstg_pool,
            tc.tile_pool(name="bias", bufs=1) as bias_pool,
            tc.tile_pool(name="xin", bufs=5) as x_pool,
            tc.tile_pool(name="oput", bufs=6) as o_pool,
            tc.tile_pool(name="psum", bufs=8, space="PSUM") as ps_pool,
        ):
            # Resident pre-scaled weights: W.T layout, k-tile t at cols
            # [t*SHARD, (t+1)*SHARD)
            wsc = wres_pool.tile([128, K_TILES * SHARD], mybir.dt.bfloat16)

            # Startup is latency-critical AND bandwidth-critical: the first
            # matmul needs W k-tile 0 plus x0/x1's first k-chunk, and every
            # byte moved during the ~50us weight stream delays later k-tiles
            # (the head's PE work is exactly arrival-paced).  So the SP ring
            # leads with k0 then quarter-tiles of x0/x1 and goes idle, the
            # ACT ring streams k-tiles 1..31 at full HBM rate, and the
            # late-needed loads (bias, x2, x3) are held back by artificial
            # WAR deps on DVE "touch" ops that fire only when late k-tiles
            # land — keeping them out of the weight-stream window.
            X_PREFETCH = min(4, m_tiles)
            x_tiles = []
            for m in range(X_PREFETCH):
                xts = x_pool.tile([128, IN_F], mybir.dt.bfloat16, tag="xts", name=f"xts{m}")
                x_tiles.append(xts)
            ones_sb = bstg_pool.tile([1, 128], mybir.dt.bfloat16, tag="ones")
            bias_row = bstg_pool.tile([1, SHARD], mybir.dt.bfloat16, tag="brow")
            nc.vector.memset(ones_sb[:], 1.0)
            nc.sync.dma_start(bias_row[:], bias[:])
            nc.sync.dma_start(wsc[:, 0:SHARD], wq[0:128, :])
            for g in range(4):
                for m in range(min(2, m_tiles)):
                    nc.sync.dma_start(
                        x_tiles[m][:, g * 1024 : (g + 1) * 1024],
                        xt[m][:, g * 8 : (g + 1) * 8, :].rearrange("p t j -> p (t j)"),
                    )

            for t in range(1, K_TILES):
                nc.scalar.dma_start(
                    wsc[:, t * SHARD : (t + 1) * SHARD],
                    wq[t * 128 : (t + 1) * 128, :],
                )

            # Bias: replicate the 3.6KB bf16 row across partitions with a K=1
            # matmul against a ones-vector (PE is idle waiting for k0 anyway)
            # instead of a 0.9MB HBM broadcast DMA inside the weight window.
            bias_sb = bias_pool.tile([128, SHARD], mybir.dt.float32)
            for ic, (n0, nw) in enumerate(N_CHUNKS):
                ps_b = ps_pool.tile([128, 512], mybir.dt.float32, tag="ps", name=f"psb{ic}")
                nc.tensor.matmul(
                    ps_b[:, :nw],
                    lhsT=ones_sb[:, 0:128],
                    rhs=bias_row[:, n0 : n0 + nw],
                    start=True,
                    stop=True,
                )
                nc.vector.tensor_copy(bias_sb[:, n0 : n0 + nw], ps_b[:, :nw])

            # Touch ops: read one column of a late W k-tile (RAW on its DMA)
            # and scribble into the target tile (WAR gates the tile's DMA).
            # The DVE queue is otherwise idle until the head drains, so x2/x3
            # chunks transfer just-in-time as the weight stream finishes,
            # keeping the bytes-bound weight window clean.
            if m_tiles > 2:
                for g, kt in enumerate((28, 29, 30, 31)):
                    nc.vector.tensor_copy(
                        x_tiles[2][:, g * 1024 : g * 1024 + 1],
                        wsc[:, kt * SHARD : kt * SHARD + 1],
                    )
                    nc.sync.dma_start(
                        x_tiles[2][:, g * 1024 : (g + 1) * 1024],
                        xt[2][:, g * 8 : (g + 1) * 8, :].rearrange("p t j -> p (t j)"),
                    )
            if m_tiles > 3:
                nc.vector.tensor_copy(
                    x_tiles[3][:, 0:1], wsc[:, 30 * SHARD : 30 * SHARD + 1]
                )
                nc.sync.dma_start(x_tiles[3][:], xt[3].rearrange("p t j -> p (t j)"))

            def finish_tile(m, n0, nw, ps):
                ot = o_pool.tile([128, 512], mybir.dt.float32, tag="ot", name=f"ot{m}_{n0}")
                nc.vector.tensor_add(ot[:, :nw], ps[:, :nw], bias_sb[:, n0 : n0 + nw])
                nc.sync.dma_start(
                    out[m * 128 : (m + 1) * 128, n0 : n0 + nw], ot[:, :nw]
                )

            # First two m-tiles in k-major order: each newly arrived W k-tile
            # immediately feeds 8 PSUM accumulation groups (~1.5us of PE work
            # per ~1.3us k-tile DMA), so the PE saturates from the start.
            m_head = min(2, m_tiles)
            head_ps = {}
            for m in range(m_head):
                for ic, (n0, nw) in enumerate(N_CHUNKS):
                    head_ps[m, ic] = ps_pool.tile(
                        [128, 512], mybir.dt.float32, tag="ps", name=f"ps{m}_{ic}"
                    )
            for t in range(K_TILES):
                for m in range(m_head):
                    for ic, (n0, nw) in enumerate(N_CHUNKS):
                        nc.tensor.matmul(
                            head_ps[m, ic][:, :nw],
                            lhsT=x_tiles[m][:, t * 128 : (t + 1) * 128],
                            rhs=wsc[:, t * SHARD + n0 : t * SHARD + n0 + nw],
                            start=(t == 0),
                            stop=(t == K_TILES - 1),
                        )
            for m in range(m_head):
                for ic, (n0, nw) in enumerate(N_CHUNKS):
                    finish_tile(m, n0, nw, head_ps[m, ic])

            # Remaining m-tiles in m-major order
            for m in range(m_head, m_tiles):
                if m < X_PREFETCH:
                    xts = x_tiles[m]
                else:
                    xts = x_pool.tile([128, IN_F], mybir.dt.bfloat16, tag="xts", name=f"xts{m}")
                    nc.sync.dma_start(xts[:], xt[m].rearrange("p t j -> p (t j)"))
                for n0, nw in N_CHUNKS:
                    ps = ps_pool.tile([128, 512], mybir.dt.float32, tag="ps")
                    for t in range(K_TILES):
                        nc.tensor.matmul(
                            ps[:, :nw],
                            lhsT=xts[:, t * 128 : (t + 1) * 128],
                            rhs=wsc[:, t * SHARD + n0 : t * SHARD + n0 + nw],
                            start=(t == 0),
                            stop=(t == K_TILES - 1),
                        )
                    ot = o_pool.tile([128, 512], mybir.dt.float32, tag="ot")
                    nc.vector.tensor_add(ot[:, :nw], ps[:, :nw], bias_sb[:, n0 : n0 + nw])
                    nc.sync.dma_start(
                        out[m * 128 : (m + 1) * 128, n0 : n0 + nw], ot[:, :nw]
                    )

    if split_waits:
        _split_multi_waits(nc)
    return nc


_PROGRAM = None


def _get_program():
    global _PROGRAM
    if _PROGRAM is None:
        _PROGRAM = _build_program()
    return _PROGRAM


def _prep_inputs(x, w_packed, w_absmax, b_packed, b_absmax):
    """Host-side marshalling: full NF4 dequant (lookup + absmax scaling),
    layout transposes, sharding."""
    # Weights: packed int32 bytes -> W.T [IN_F, OUT_F] of scaled values.
    b = np.asarray(w_packed).astype(np.uint8).reshape(OUT_F, IN_F // 2)
    bT = np.ascontiguousarray(b.T)  # [2048, 14336]
    valsT = np.empty((IN_F, OUT_F), dtype=np.float32)
    valsT[0::2] = NF4[bT >> 4]
    valsT[1::2] = NF4[bT & 15]

    # Per-64-block absmax scaling: absmax[n, k//64] applied along k.
    am = np.asarray(w_absmax, dtype=np.float32).reshape(OUT_F, IN_F // BLOCK)
    wT = (
        valsT.reshape(IN_F // BLOCK, BLOCK, OUT_F) * am.T[:, None, :]
    ).reshape(IN_F, OUT_F).astype(BF16)

    # x: [M, K] f32 -> bf16 tiles [m_tile, p(k%128), k_tile, j(m%128)]
    xbf = np.asarray(x, dtype=np.float32).astype(BF16)
    xt5 = np.ascontiguousarray(
        xbf.reshape(M_TILES, 128, K_TILES, 128).transpose(0, 3, 2, 1)
    )

    # Bias: full dequant on host (14336 elements — negligible)
    bb = np.asarray(b_packed).astype(np.uint8)
    bcodes = np.empty(OUT_F, dtype=np.uint8)
    bcodes[0::2] = bb >> 4
    bcodes[1::2] = bb & 15
    bias_full = (
        NF4[bcodes].reshape(-1, BLOCK)
        * np.asarray(b_absmax, dtype=np.float32).reshape(-1, 1)
    ).reshape(OUT_F)

    in_maps = []
    for c in range(N_CORES):
        n0, n1 = c * SHARD, (c + 1) * SHARD
        in_maps.append(
            {
                "wq": np.ascontiguousarray(wT[:, n0:n1]),
                "xt": xt5,
                "bias": np.ascontiguousarray(bias_full[n0:n1]).astype(BF16).reshape(1, SHARD),
            }
        )
    return in_maps


def kernel(x, w_packed, w_absmax, b_packed, b_absmax, trace=False, **run_kwargs):
    nc = _get_program()
    in_maps = _prep_inputs(x, w_packed, w_absmax, b_packed, b_absmax)
    res = run_bass_kernel_spmd(
        nc, in_maps, core_ids=list(range(N_CORES)), trace=trace, **run_kwargs
    )
    out = np.concatenate([res.results[c]["out"] for c in range(N_CORES)], axis=1)
    kernel.last_results = res
    return out
